# revision 31
# baseline (speedup 1.0000x reference)
"""MultiHeadGAT Trainium2 kernel: 8-core batch-parallel, transposed-layout pipeline.

Math: for scores e = lrelu(s_i[n] + s_j[m]), softmax numerator
  p = exp(lrelu(s_i+s_j)) = e^{0.2 s_i} * max(e^{0.8 s_i} * e^{s_j}, e^{0.2 s_j})
The e^{0.2 s_i} row factor cancels in softmax, so on-device we only compute
  q[m, n] = adjT[m, n] * max(Wbc[m, n] * u[m], v[m])
with Wbc = broadcast(e^{0.8 s_i}) (n-varying), u = e^{s_j}, v = e^{0.2 s_j}
(per-partition scalars), which is one fused tensor_scalar (mult+max) plus one
tensor_tensor (mask) per tile. Attention output and row-sum Z come from one
PE matmul with lhsT = [ones | pad | Wh_head]; normalization 1/Z = exp(-ln(Z)).
"""

import sys

sys.path.insert(0, "/opt/trn_rl_repo")

import numpy as np

B, N, IN_DIM, H, HD = 8, 1024, 128, 8, 16
OUT_DIM = H * HD
EPS = 1e-5
NB = N // 128  # 8 m-blocks

_CACHE = {}


def _patch_act_tables():
    # Force one activation table set for the whole kernel: every function we
    # use (Exp, Ln, Copy, Square, Relu, Identity) lives in
    # natural_log_exp_and_others; emptying the other sets makes Bacc's
    # table-load inserter emit exactly one ACT_TABLE_LOAD instead of
    # thrashing between exp/ln/small sets (~2.5us per reload).
    import concourse.bacc as bacc
    import concourse.hw_specs as hw_specs
    if getattr(bacc, "_act_tables_patched", False):
        return
    orig = hw_specs.get_activation_tables

    def patched(arch):
        t = dict(orig(arch))
        keep = "natural_log_exp_and_others"
        return {k: (v if k == keep else set()) for k, v in t.items()}

    bacc.get_activation_tables = patched
    bacc._act_tables_patched = True


_QMASK_NAME = "QMASK_ANT"
_QMASK_STATE = {}


def _qmask_register(ver):
    """Custom fused DVE op: out = max(in0*s0, s1) * in1, with a hand-authored
    2x_1P uop program (two packed 16-bit elements per cycle)."""
    if _QMASK_NAME in _QMASK_STATE:
        return _QMASK_STATE[_QMASK_NAME]
    import concourse.dve_ops as dops
    from concourse.dve_spec import Spec, Src0, Src1, C0, C1, maxx, lower
    from concourse.dve_uop import (
        DveOpSpec, UopConfig, UopDpConfig, InpSel, AluInp, DelayInp,
        OutPath, OutSel, AluOp, Trigger,
    )

    spec = Spec(
        body=maxx(Src0 * C0, C1) * Src1,
        reference=lambda in0, in1, s0, s1, imm2: (
            np.maximum(in0 * s0, s1) * in1
        ).astype(np.float32),
    )
    op = dops.DveOp(name=_QMASK_NAME, spec=spec, subdim=False, uops_sha={})
    if all(o.name != _QMASK_NAME for o in dops.OPS):
        dops.OPS.append(op)
    dops.CUSTOM_DVE_SPECS[_QMASK_NAME] = spec
    if _QMASK_NAME not in dops._SUB_OPCODE_FOR_NAME:
        row = max(dops._SUB_OPCODE_FOR_NAME.values()) + 1
        assert row < 0x20
        dops._SUB_OPCODE_FOR_NAME[_QMASK_NAME] = row
    row = dops._SUB_OPCODE_FOR_NAME[_QMASK_NAME]

    # 2x_1P program: lo chain blk0-2 (SRC_0*C0 max C1 * SRC_1), hi chain
    # blk3-5 on the packed hi halves; lo result rides delay line 0 from blk3.
    u = UopConfig()
    u.enable_input(InpSel.SRC_0, 1)
    u.enable_input(InpSel.CONST_0, 2)
    u.enable_input(InpSel.CONST_1, 3)
    u.enable_input(InpSel.SRC_1, 4)
    u.enable_input(InpSel.SRC_0_HI, 5)
    u.enable_input(InpSel.SRC_1_HI, 6)
    u.require_inp0 = 1
    u.require_inp1 = 1
    u.trigger = (Trigger.SRC_TENSOR_DONE, Trigger.NONE, Trigger.NONE)
    u.next_uop = (0, 0, 0)
    u.out = {
        OutPath.WR0_LO: OutSel.DELAY_0,
        OutPath.WR0_HI: OutSel.ALU_OUT,
        OutPath.WR1_LO: OutSel.ALU_OUT,
        OutPath.WR1_HI: OutSel.ALU_OUT,
    }
    u.out_enable = {OutPath.WR0_LO: 1, OutPath.WR0_HI: 1,
                    OutPath.WR1_LO: 0, OutPath.WR1_HI: 0}
    CARRY = [DelayInp.PREV_DELAY] * 7

    def blk(aop, s0, s1, delay=None):
        return UopDpConfig(
            op=aop, alu_src0=s0, alu_src1=s1,
            delay=list(delay if delay is not None else CARRY),
            alu_out_enable=1,
            delay_enable=[1, 1, 1, 1, 1, 1, 0],
        )

    dp = [
        blk(AluOp.MULTIPLY, AluInp.PREV_DELAY_0, AluInp.PREV_DELAY_1),
        blk(AluOp.MAX, AluInp.PREV_ALU_OUT, AluInp.PREV_DELAY_2),
        blk(AluOp.MULTIPLY, AluInp.PREV_ALU_OUT, AluInp.PREV_DELAY_3),
        blk(AluOp.MULTIPLY, AluInp.PREV_DELAY_4, AluInp.PREV_DELAY_1,
            delay=[DelayInp.PREV_ALU_OUT] + [DelayInp.PREV_DELAY] * 6),
        blk(AluOp.MAX, AluInp.PREV_ALU_OUT, AluInp.PREV_DELAY_2),
        blk(AluOp.MULTIPLY, AluInp.PREV_ALU_OUT, AluInp.PREV_DELAY_5),
        blk(AluOp.BYPASS, AluInp.PREV_ALU_OUT, AluInp.PREV_ALU_OUT),
        blk(AluOp.BYPASS, AluInp.PREV_ALU_OUT, AluInp.PREV_ALU_OUT),
    ]
    u.datapath_config = dp

    u1x = lower(spec, ver=ver)
    compiled = DveOpSpec(
        name=_QMASK_NAME, opcode=row, uops=u1x, uops_2x=[u],
        perf_max=1, rd1_en=True,
    )
    compiled.validate(ver)
    dops._COMPILE_CACHE[(_QMASK_NAME, ver)] = compiled
    _QMASK_STATE[_QMASK_NAME] = op
    return op


def _qmask_emit(nc, out, in0, s0, s1, in1):
    """out = max(in0*s0, s1) * in1 (s0/s1 per-partition [P,1] APs)."""
    from concourse.bass import dve_ver_for
    from concourse import bass_isa, mybir
    import concourse.dve_ops as dops

    ver = dve_ver_for(nc.trn_type)
    op = _qmask_register(ver)
    vec = nc.vector
    if op.name not in vec.bass.m.ant_custom_dve_ops:
        vec.bass.m.ant_custom_dve_ops = sorted(
            {*vec.bass.m.ant_custom_dve_ops, op.name}
        )
    shape = bass_isa.CustomDveShape.TTSS
    isa_opcode = vec.bass.isa.Opcode[
        f"NEURON_ISA_TPB_OPCODE_CUSTOM_DVE_ANT_{shape.slot()}"
    ].value
    ins = [
        vec.lower_ap(in0, for_isa=True, opt=True),
        vec.lower_ap(in1, for_isa=True, opt=True),
        vec.lower_ap(s0, for_isa=True),
        vec.lower_ap(s1, for_isa=True),
    ]
    outs = [vec.lower_ap(out, for_isa=True, opt=True)]
    return vec.add_instruction(
        bass_isa.InstCustomDveAnt(
            name=vec.bass.get_next_instruction_name(),
            op_name=op.name, rd1_en=True, subdim=0, imm2=0.0,
            shape=shape, row=dops._SUB_OPCODE_FOR_NAME[_QMASK_NAME],
            isa_opcode=isa_opcode, perf_max=1, ins=ins, outs=outs,
        )
    )


_VARSQ_NAME = "VARSQ_ANT"


def _varsq_register(ver):
    if _VARSQ_NAME in _QMASK_STATE:
        return _QMASK_STATE[_VARSQ_NAME]
    import concourse.dve_ops as dops
    from concourse.dve_spec import Spec, Src0, Src1, lower, sq

    spec = Spec(
        body=Src0 - sq(Src1),
        reference=lambda in0, in1, s0, s1, imm2: (
            in0 - in1 * in1
        ).astype(np.float32),
    )
    op = dops.DveOp(name=_VARSQ_NAME, spec=spec, subdim=False, uops_sha={})
    if all(o.name != _VARSQ_NAME for o in dops.OPS):
        dops.OPS.append(op)
    dops.CUSTOM_DVE_SPECS[_VARSQ_NAME] = spec
    if _VARSQ_NAME not in dops._SUB_OPCODE_FOR_NAME:
        row = max(dops._SUB_OPCODE_FOR_NAME.values()) + 1
        assert row < 0x20
        dops._SUB_OPCODE_FOR_NAME[_VARSQ_NAME] = row
    row = dops._SUB_OPCODE_FOR_NAME[_VARSQ_NAME]
    from concourse.dve_uop import DveOpSpec
    compiled = DveOpSpec(
        name=_VARSQ_NAME, opcode=row, uops=lower(spec, ver=ver),
        perf_max=0, rd1_en=True,
    )
    compiled.validate(ver)
    dops._COMPILE_CACHE[(_VARSQ_NAME, ver)] = compiled
    _QMASK_STATE[_VARSQ_NAME] = op
    return op


def _varsq_emit(nc, out, in0, in1):
    """out = in0 - in1*in1 (in0 may be PSUM)."""
    from concourse.bass import dve_ver_for
    from concourse import bass_isa, mybir
    import concourse.dve_ops as dops

    ver = dve_ver_for(nc.trn_type)
    op = _varsq_register(ver)
    vec = nc.vector
    if op.name not in vec.bass.m.ant_custom_dve_ops:
        vec.bass.m.ant_custom_dve_ops = sorted(
            {*vec.bass.m.ant_custom_dve_ops, op.name}
        )
    shape = bass_isa.CustomDveShape.TTSS
    isa_opcode = vec.bass.isa.Opcode[
        f"NEURON_ISA_TPB_OPCODE_CUSTOM_DVE_ANT_{shape.slot()}"
    ].value
    zero = mybir.ImmediateValue(dtype=mybir.dt.float32, value=0.0)
    ins = [
        vec.lower_ap(in0, for_isa=True, opt=True),
        vec.lower_ap(in1, for_isa=True, opt=True),
        zero, zero,
    ]
    outs = [vec.lower_ap(out, for_isa=True, opt=True)]
    return vec.add_instruction(
        bass_isa.InstCustomDveAnt(
            name=vec.bass.get_next_instruction_name(),
            op_name=op.name, rd1_en=True, subdim=0, imm2=0.0,
            shape=shape, row=dops._SUB_OPCODE_FOR_NAME[_VARSQ_NAME],
            isa_opcode=isa_opcode, perf_max=0, ins=ins, outs=outs,
        )
    )


_RELUB_NAME = "RELUB_ANT"


def _relub_register(ver):
    if _RELUB_NAME in _QMASK_STATE:
        return _QMASK_STATE[_RELUB_NAME]
    import concourse.dve_ops as dops
    from concourse.dve_spec import Spec, Src0, C0, lower, relu

    spec = Spec(
        body=relu(Src0 + C0),
        reference=lambda in0, in1, s0, s1, imm2: np.maximum(
            in0 + s0, 0.0
        ).astype(np.float32),
    )
    op = dops.DveOp(name=_RELUB_NAME, spec=spec, subdim=False, uops_sha={})
    if all(o.name != _RELUB_NAME for o in dops.OPS):
        dops.OPS.append(op)
    dops.CUSTOM_DVE_SPECS[_RELUB_NAME] = spec
    if _RELUB_NAME not in dops._SUB_OPCODE_FOR_NAME:
        row = max(dops._SUB_OPCODE_FOR_NAME.values()) + 1
        assert row < 0x20
        dops._SUB_OPCODE_FOR_NAME[_RELUB_NAME] = row
    row = dops._SUB_OPCODE_FOR_NAME[_RELUB_NAME]
    from concourse.dve_uop import DveOpSpec
    compiled = DveOpSpec(
        name=_RELUB_NAME, opcode=row, uops=lower(spec, ver=ver),
        perf_max=0, rd1_en=False,
    )
    compiled.validate(ver)
    dops._COMPILE_CACHE[(_RELUB_NAME, ver)] = compiled
    _QMASK_STATE[_RELUB_NAME] = op
    return op


def _relub_emit(nc, out, in0, s0):
    """out = relu(in0 + s0) on DVE (in0 may be PSUM; s0 per-partition AP)."""
    from concourse.bass import dve_ver_for
    from concourse import bass_isa, mybir
    import concourse.dve_ops as dops

    ver = dve_ver_for(nc.trn_type)
    op = _relub_register(ver)
    vec = nc.vector
    if op.name not in vec.bass.m.ant_custom_dve_ops:
        vec.bass.m.ant_custom_dve_ops = sorted(
            {*vec.bass.m.ant_custom_dve_ops, op.name}
        )
    shape = bass_isa.CustomDveShape.TTSS
    isa_opcode = vec.bass.isa.Opcode[
        f"NEURON_ISA_TPB_OPCODE_CUSTOM_DVE_ANT_{shape.slot()}"
    ].value
    zero = mybir.ImmediateValue(dtype=mybir.dt.float32, value=0.0)
    ins = [
        vec.lower_ap(in0, for_isa=True, opt=True),
        vec.lower_ap(s0, for_isa=True),
        zero,
    ]
    outs = [vec.lower_ap(out, for_isa=True, opt=True)]
    return vec.add_instruction(
        bass_isa.InstCustomDveAnt(
            name=vec.bass.get_next_instruction_name(),
            op_name=op.name, rd1_en=False, subdim=0, imm2=0.0,
            shape=shape, row=dops._SUB_OPCODE_FOR_NAME[_RELUB_NAME],
            isa_opcode=isa_opcode, perf_max=0, ins=ins, outs=outs,
        )
    )


def _build_program():
    import concourse.bacc as bacc
    import concourse.mybir as mybir
    import concourse.tile as tile

    _patch_act_tables()

    F16 = mybir.dt.float16
    F32 = mybir.dt.float32
    AF = mybir.ActivationFunctionType
    OP = mybir.AluOpType

    nc = bacc.Bacc("TRN2", target_bir_lowering=False, debug=False, num_devices=8)

    # ---- I/O ----
    hT = nc.dram_tensor("hT", [128, N], F16, kind="ExternalInput")
    adjT = nc.dram_tensor("adjT", [128, NB * N], F16, kind="ExternalInput")
    # critical pack: [wcat 128 | wadst 8 | wasrep 1024]
    wpackA = nc.dram_tensor("wpackA", [128, 1160], F16, kind="ExternalInput")
    # late pack: [w1 256 | w2 256]
    wpackB = nc.dram_tensor("wpackB", [128, 512], F16, kind="ExternalInput")
    augpk = nc.dram_tensor("augpk", [128, 3072], F16, kind="ExternalInput")
    # packed f32 cols: [b1c 2 | b2c 1 | g1 1 | b1l 1 | g2 1 | b2l 1 | zbias 1 | eps 1]
    wpack32 = nc.dram_tensor("wpack32", [128, 9], F32, kind="ExternalInput")
    sel = nc.dram_tensor("sel", [16, H * 128], F16, kind="ExternalInput")
    e16cat = nc.dram_tensor("e16cat", [1, H * 128], F16, kind="ExternalInput")
    outT = nc.dram_tensor("outT", [128, N], F16, kind="ExternalOutput")

    with tile.TileContext(nc) as tc:
        with (
            tc.tile_pool(name="const", bufs=1) as cpool,
            tc.tile_pool(name="big", bufs=1) as big,
            tc.tile_pool(name="work", bufs=2) as work,
            tc.tile_pool(name="mid", bufs=1) as mid,
            tc.tile_pool(name="rows", bufs=1) as rows,
        ):
            # ---- load everything ----
            # All on the SP (sync) HWDGE ring: FIFO order = priority order.
            hT_t = cpool.tile([128, N], F16)
            nc.sync.dma_start(hT_t[:], hT[:])
            wpA = cpool.tile([128, 1160], F16)
            nc.sync.dma_start(wpA[:], wpackA[:])
            adjq = [
                cpool.tile([128, 4 * N], F16, tag=f"adj{i}", name=f"adj{i}")
                for i in range(2)
            ]
            nc.sync.dma_start(adjq[0][:, 0:2 * N], adjT[:, 0:2 * N])
            nc.sync.dma_start(adjq[0][:, 2 * N:4 * N], adjT[:, 2 * N:4 * N])
            augt = cpool.tile([128, 3072], F16)
            nc.sync.dma_start(augt[:], augpk[:])
            nc.sync.dma_start(adjq[1][:, 0:2 * N], adjT[:, 4 * N:6 * N])
            nc.sync.dma_start(adjq[1][:, 2 * N:4 * N], adjT[:, 6 * N:8 * N])
            wpB = cpool.tile([128, 512], F16)
            nc.sync.dma_start(wpB[:], wpackB[:])
            wp32 = cpool.tile([128, 9], F32)
            nc.sync.dma_start(wp32[:], wpack32[:])
            sel_t = cpool.tile([16, H * 128], F16)
            nc.sync.dma_start(sel_t[:], sel[:])
            e16cat_t = cpool.tile([1, H * 128], F16)
            nc.sync.dma_start(e16cat_t[:], e16cat[:])

            wcat_t = wpA[:, 0:128]
            wadst_t = wpA[:, 128:136]
            wasrep_t = wpA[:, 136:1160]
            w1_t = wpB[:, 0:256]
            w2_t = wpB[:, 256:512]
            aug = augt[:]
            b1_t = wp32[:, 0:2]
            b2_t = wp32[:, 2:3]
            g1_t = wp32[:, 3:4]
            b1l_t = wp32[:, 4:5]
            g2_t = wp32[:, 5:6]
            b2l_t = wp32[:, 6:7]
            zbias = wp32[:, 7:8]
            epsbias = wp32[:, 8:9]

            onescol = cpool.tile([128, 1], F16)
            nc.vector.memset(onescol[:], 1.0)
            jmat = cpool.tile([128, 128], F16)
            nc.vector.memset(jmat[:], 1.0 / 128)
            onesrow = cpool.tile([1, 128], F32)
            nc.vector.memset(onesrow[:], 1.0)

            # ---- phase 1: s-cols(u,v), Wbc, Wh_nat->aug ----
            u_t = [big.tile([128, H], F32, tag=f"u{i}", name=f"u{i}") for i in range(NB)]
            v_t = [big.tile([128, H], F32, tag=f"v{i}", name=f"v{i}") for i in range(NB)]
            wbc = [big.tile([128, N], F16, tag=f"wbc{i}", name=f"wbc{i}") for i in range(H)]
            aug4w = aug.rearrange("p (m h c) -> p m h c", m=NB, h=H, c=48)

            with tc.tile_pool(name="ps1", bufs=3, space="PSUM") as ps1:
                for mb in range(NB):
                    sc_ps = ps1.tile([128, H], F32, tag="ps1")
                    nc.tensor.matmul(
                        sc_ps[:], hT_t[:, mb * 128:(mb + 1) * 128], wadst_t,
                        start=True, stop=True,
                    )
                    nc.scalar.activation(u_t[mb][:], sc_ps[:], AF.Exp, scale=1.0)
                    nc.scalar.activation(v_t[mb][:], sc_ps[:], AF.Exp, scale=0.2)
                    if mb == 0:
                        wb_ps = ps1.tile([128, N], F32, tag="ps1")
                        for ch in range(2):
                            nc.tensor.matmul(
                                wb_ps[:, ch * 512:(ch + 1) * 512],
                                wasrep_t[:, 0:128],
                                hT_t[:, ch * 512:(ch + 1) * 512],
                                start=True, stop=True,
                            )
                        nc.scalar.activation(wbc[0][:], wb_ps[:], AF.Exp, scale=0.8)
                for hh in range(1, H):
                    wb_ps = ps1.tile([128, N], F32, tag="ps1")
                    for ch in range(2):
                        nc.tensor.matmul(
                            wb_ps[:, ch * 512:(ch + 1) * 512],
                            wasrep_t[:, hh * 128:(hh + 1) * 128],
                            hT_t[:, ch * 512:(ch + 1) * 512],
                            start=True, stop=True,
                        )
                    nc.scalar.activation(wbc[hh][:], wb_ps[:], AF.Exp, scale=0.8)
                for mb in range(NB):
                    wn_ps = ps1.tile([128, 128], F32, tag="ps1")
                    nc.tensor.matmul(
                        wn_ps[:], hT_t[:, mb * 128:(mb + 1) * 128], wcat_t,
                        start=True, stop=True,
                    )
                    wn4 = wn_ps[:].rearrange("p (h d) -> p h d", h=H, d=16)
                    nc.scalar.activation(aug4w[:, mb, :, 32:48], wn4[:], AF.Copy)

            # ---- phase 2: attention ----
            stage_all = big.tile([16, H * N], F16)
            with (
                tc.tile_pool(name="ps48", bufs=2, space="PSUM") as ps48,
                tc.tile_pool(name="psacc", bufs=1, space="PSUM") as psacc,
                tc.tile_pool(name="psz", bufs=1, space="PSUM") as psz,
            ):
                asm_ps = psacc.tile([128, N], F32)
                zbc_ps = psz.tile([128, N], F32)
                for hh in range(H):
                    q_half = [
                        work.tile([128, 4 * N], F16, tag="qa", name="qa"),
                        work.tile([128, 4 * N], F16, tag="qb", name="qb"),
                    ]
                    at_ps = ps48.tile([48, N], F32)
                    for half in range(2):
                        qh = q_half[half]
                        for mb in range(half * 4, half * 4 + 4):
                            _qmask_emit(
                                nc, qh[:, (mb % 4) * N:(mb % 4 + 1) * N],
                                wbc[hh][:],
                                u_t[mb][:, hh: hh + 1], v_t[mb][:, hh: hh + 1],
                                adjq[half][:, (mb % 4) * N:(mb % 4 + 1) * N],
                            )
                        for mb in range(half * 4, half * 4 + 4):
                            for ch in range(2):
                                nc.tensor.matmul(
                                    at_ps[:, ch * 512:(ch + 1) * 512],
                                    aug[:, mb * 384 + hh * 48: mb * 384 + hh * 48 + 48],
                                    qh[:, (mb % 4) * N + ch * 512: (mb % 4) * N + ch * 512 + 512],
                                    start=(mb == 0), stop=(mb == NB - 1),
                                )
                    lnz = work.tile([1, N], F32, tag="lnz")
                    nc.scalar.activation(lnz[:], at_ps[0:1, :], AF.Ln, bias=zbias[0:1])
                    zinv_h = work.tile([1, N], F16, tag="zinvh")
                    nc.scalar.activation(zinv_h[:], lnz[:], AF.Exp, scale=-1.0)
                    nc.scalar.activation(
                        stage_all[:, hh * N:(hh + 1) * N], at_ps[32:48, :], AF.Copy
                    )
                    for ch in range(2):
                        nc.tensor.matmul(
                            zbc_ps[:, ch * 512:(ch + 1) * 512],
                            e16cat_t[0:1, hh * 128:(hh + 1) * 128],
                            zinv_h[0:1, ch * 512:(ch + 1) * 512],
                            start=(hh == 0), stop=(hh == H - 1),
                        )
                        nc.tensor.matmul(
                            asm_ps[:, ch * 512:(ch + 1) * 512],
                            sel_t[:, hh * 128:(hh + 1) * 128],
                            stage_all[:, hh * N + ch * 512: hh * N + ch * 512 + 512],
                            start=(hh == 0), stop=(hh == H - 1),
                        )

                zbcf = big.tile([128, N], F16)
                nc.scalar.activation(zbcf[:, 0:512], zbc_ps[:, 0:512], AF.Copy)
                nc.scalar.activation(zbcf[:, 512:1024], zbc_ps[:, 512:1024],
                                     AF.Copy)
                hh_t = big.tile([128, N], F16)
                x_res = big.tile([128, N], F16)
                for c2 in range(2):
                    nc.vector.tensor_tensor(
                        hh_t[:, c2 * 512:(c2 + 1) * 512],
                        asm_ps[:, c2 * 512:(c2 + 1) * 512],
                        zbcf[:, c2 * 512:(c2 + 1) * 512], op=OP.mult)
                    nc.vector.tensor_tensor(
                        x_res[:, c2 * 512:(c2 + 1) * 512],
                        hh_t[:, c2 * 512:(c2 + 1) * 512],
                        hT_t[:, c2 * 512:(c2 + 1) * 512], op=OP.add)

            with tc.tile_pool(name="ps3", bufs=2, space="PSUM") as ps3:
                # ---- chunked epilogue: normalize+residual, LN1, FFN, LN2 ----
                C = 512

                def cs(t, c):
                    return t[:, c * C:(c + 1) * C]

                def layernorm_T(x_in, g_col, b_col, out_tile, ps_pool, nm):
                    """Column-chunked transposed layernorm; J=ones/128 matmul
                    produces mean / mean-square directly as broadcast tiles."""
                    x2 = mid.tile([128, N], F16, tag=f"x2{nm}")
                    for c in range(N // C):
                        nc.vector.tensor_tensor(
                            cs(x2, c), cs(x_in, c), cs(x_in, c), op=OP.mult
                        )
                    for c in range(N // C):
                        mu_ps = ps_pool.tile([128, C], F32, tag="psb")
                        ssq_ps = ps_pool.tile([128, C], F32, tag="psb")
                        nc.tensor.matmul(mu_ps[:], jmat[:], cs(x_in, c),
                                         start=True, stop=True)
                        nc.tensor.matmul(ssq_ps[:], jmat[:], cs(x2, c),
                                         start=True, stop=True)
                        mu_bc = mid.tile([128, C], F16, tag=f"mbc{nm}{c}")
                        nc.scalar.activation(mu_bc[:], mu_ps[:], AF.Copy)
                        var = mid.tile([128, C], F16, tag=f"va{nm}{c}")
                        _varsq_emit(nc, var[:], ssq_ps[:], mu_bc[:])
                        lnv = mid.tile([128, C], F16, tag=f"lv{nm}{c}")
                        nc.scalar.activation(lnv[:], var[:], AF.Ln, bias=epsbias)
                        rstd = mid.tile([128, C], F16, tag=f"rs{nm}{c}")
                        nc.scalar.activation(rstd[:], lnv[:], AF.Exp, scale=-0.5)
                        t_ = mid.tile([128, C], F16, tag=f"lnt{nm}{c}")
                        nc.vector.tensor_tensor(t_[:], cs(x_in, c), mu_bc[:],
                                                op=OP.subtract)
                        xn = mid.tile([128, C], F16, tag=f"lnxn{nm}{c}")
                        nc.vector.tensor_tensor(xn[:], t_[:], rstd[:], op=OP.mult)
                        nc.vector.tensor_scalar(
                            cs(out_tile, c), xn[:], g_col[:], b_col[:],
                            op0=OP.mult, op1=OP.add,
                        )

                xc = big.tile([128, N], F16)
                y1s = big.tile([128, 2 * N], F16)
                y2b = big.tile([128, N], F16)
                z_res = big.tile([128, N], F16)
                outT_sb = big.tile([128, N], F16)

                layernorm_T(x_res, g1_t, b1l_t, xc, ps3, "a")

                # FFN (chunked)
                for cb in range(2):
                    y1_ps = ps3.tile([128, N], F32, tag="ps3")
                    for c in range(N // C):
                        nc.tensor.matmul(
                            cs(y1_ps, c), w1_t[:, cb * 128:(cb + 1) * 128],
                            cs(xc, c), start=True, stop=True,
                        )
                        nc.scalar.activation(
                            y1s[:, cb * N + c * C: cb * N + (c + 1) * C],
                            cs(y1_ps, c), AF.Relu, bias=b1_t[:, cb:cb + 1],
                        )
                y2_ps = ps3.tile([128, N], F32, tag="ps3")
                for cb in range(2):
                    for c in range(N // C):
                        nc.tensor.matmul(
                            cs(y2_ps, c), w2_t[:, cb * 128:(cb + 1) * 128],
                            y1s[:, cb * N + c * C: cb * N + (c + 1) * C],
                            start=(cb == 0), stop=(cb == 1),
                        )
                for c in range(N // C):
                    nc.vector.scalar_tensor_tensor(
                        cs(z_res, c), cs(y2_ps, c), b2_t, cs(xc, c),
                        op0=OP.add, op1=OP.add,
                    )
                layernorm_T(z_res, g2_t, b2l_t, outT_sb, ps3, "b")
                for c in range(N // C):
                    nc.sync.dma_start(outT[:, c * C:(c + 1) * C],
                                      outT_sb[:, c * C:(c + 1) * C])

    nc.compile()
    return nc


def _host_prep(h, adj_mask, W, a, ln1_g, ln1_b, w1, b1, w2, b2, ln2_g, ln2_b):
    f16 = np.float16
    f32 = np.float32
    wcat = np.ascontiguousarray(
        np.transpose(np.asarray(W, f32), (1, 0, 2)).reshape(128, 128)
    ).astype(f16)
    a = np.asarray(a, f32)
    a_src, a_dst = a[:, :HD], a[:, HD:]
    Wf = np.asarray(W, f32)
    wa_dst = np.einsum("hid,hd->ih", Wf, a_dst).astype(f16)
    wa_src = np.einsum("hid,hd->ih", Wf, a_src)
    wasrep = np.repeat(wa_src[:, :, None], 128, axis=2).reshape(128, H * 128).astype(f16)
    sel = np.zeros((16, H * 128), f16)
    for hh in range(H):
        sel[np.arange(16), hh * 128 + hh * 16 + np.arange(16)] = 1.0
    e16cat = np.zeros((1, H * 128), f16)
    for hh in range(H):
        e16cat[0, hh * 128 + hh * 16: hh * 128 + (hh + 1) * 16] = 1.0
    w1c = np.asarray(w1, f32).astype(f16)
    w2c = np.ascontiguousarray(
        np.asarray(w2, f32).reshape(2, 128, 128).transpose(1, 0, 2).reshape(128, 256)
    ).astype(f16)
    augs = np.zeros((128, NB * 384), f16)
    augs[:, np.arange(NB * H) * 48] = 1.0  # ones columns
    wpackA = np.concatenate([wcat, wa_dst, wasrep], axis=1)
    wpackB = np.concatenate([w1c, w2c], axis=1)

    wpack32 = np.zeros((128, 9), f32)
    wpack32[:, 0:2] = np.asarray(b1, f32).reshape(2, 128).T
    wpack32[:, 2] = np.asarray(b2, f32)
    wpack32[:, 3] = np.asarray(ln1_g, f32)
    wpack32[:, 4] = np.asarray(ln1_b, f32)
    wpack32[:, 5] = np.asarray(ln2_g, f32)
    wpack32[:, 6] = np.asarray(ln2_b, f32)
    wpack32[:, 7] = 1e-4
    wpack32[:, 8] = EPS

    shared = dict(wpackA=wpackA, wpackB=wpackB, augpk=augs, wpack32=wpack32,
                  sel=sel, e16cat=e16cat)

    h = np.asarray(h, f32)
    adj = np.asarray(adj_mask)
    in_maps = []
    for b in range(B):
        hT = np.ascontiguousarray(h[b].T).astype(f16)
        adjT = np.ascontiguousarray(
            (adj[b] != 0).T.astype(f16).reshape(NB, 128, N).transpose(1, 0, 2).reshape(128, NB * N)
        )
        in_maps.append(dict(hT=hT, adjT=adjT, **shared))
    return in_maps


def kernel(**inputs):
    from concourse.bass_utils import run_bass_kernel_spmd

    if "nc" not in _CACHE:
        _CACHE["nc"] = _build_program()
    nc = _CACHE["nc"]

    in_maps = _host_prep(**inputs)
    res = run_bass_kernel_spmd(nc, in_maps, list(range(B)))
    out = np.empty((B, N, OUT_DIM), np.float32)
    for b in range(B):
        out[b] = res.results[b]["outT"].T
    return out



# revision 32
# speedup vs baseline: 1.0120x; 1.0120x over previous
"""MultiHeadGAT Trainium2 kernel: 8-core batch-parallel, transposed-layout pipeline.

Math: for scores e = lrelu(s_i[n] + s_j[m]), softmax numerator
  p = exp(lrelu(s_i+s_j)) = e^{0.2 s_i} * max(e^{0.8 s_i} * e^{s_j}, e^{0.2 s_j})
The e^{0.2 s_i} row factor cancels in softmax, so on-device we only compute
  q[m, n] = adjT[m, n] * max(Wbc[m, n] * u[m], v[m])
with Wbc = broadcast(e^{0.8 s_i}) (n-varying), u = e^{s_j}, v = e^{0.2 s_j}
(per-partition scalars), which is one fused tensor_scalar (mult+max) plus one
tensor_tensor (mask) per tile. Attention output and row-sum Z come from one
PE matmul with lhsT = [ones | pad | Wh_head]; normalization 1/Z = exp(-ln(Z)).
"""

import sys

sys.path.insert(0, "/opt/trn_rl_repo")

import numpy as np

B, N, IN_DIM, H, HD = 8, 1024, 128, 8, 16
OUT_DIM = H * HD
EPS = 1e-5
NB = N // 128  # 8 m-blocks

_CACHE = {}


def _patch_act_tables():
    # Force one activation table set for the whole kernel: every function we
    # use (Exp, Ln, Copy, Square, Relu, Identity) lives in
    # natural_log_exp_and_others; emptying the other sets makes Bacc's
    # table-load inserter emit exactly one ACT_TABLE_LOAD instead of
    # thrashing between exp/ln/small sets (~2.5us per reload).
    import concourse.bacc as bacc
    import concourse.hw_specs as hw_specs
    if getattr(bacc, "_act_tables_patched", False):
        return
    orig = hw_specs.get_activation_tables

    def patched(arch):
        t = dict(orig(arch))
        keep = "natural_log_exp_and_others"
        return {k: (v if k == keep else set()) for k, v in t.items()}

    bacc.get_activation_tables = patched
    bacc._act_tables_patched = True


_QMASK_NAME = "QMASK_ANT"
_QMASK_STATE = {}


def _qmask_register(ver):
    """Custom fused DVE op: out = max(in0*s0, s1) * in1, with a hand-authored
    2x_1P uop program (two packed 16-bit elements per cycle)."""
    if _QMASK_NAME in _QMASK_STATE:
        return _QMASK_STATE[_QMASK_NAME]
    import concourse.dve_ops as dops
    from concourse.dve_spec import Spec, Src0, Src1, C0, C1, maxx, lower
    from concourse.dve_uop import (
        DveOpSpec, UopConfig, UopDpConfig, InpSel, AluInp, DelayInp,
        OutPath, OutSel, AluOp, Trigger,
    )

    spec = Spec(
        body=maxx(Src0 * C0, C1) * Src1,
        reference=lambda in0, in1, s0, s1, imm2: (
            np.maximum(in0 * s0, s1) * in1
        ).astype(np.float32),
    )
    op = dops.DveOp(name=_QMASK_NAME, spec=spec, subdim=False, uops_sha={})
    if all(o.name != _QMASK_NAME for o in dops.OPS):
        dops.OPS.append(op)
    dops.CUSTOM_DVE_SPECS[_QMASK_NAME] = spec
    if _QMASK_NAME not in dops._SUB_OPCODE_FOR_NAME:
        row = max(dops._SUB_OPCODE_FOR_NAME.values()) + 1
        assert row < 0x20
        dops._SUB_OPCODE_FOR_NAME[_QMASK_NAME] = row
    row = dops._SUB_OPCODE_FOR_NAME[_QMASK_NAME]

    # 2x_1P program: lo chain blk0-2 (SRC_0*C0 max C1 * SRC_1), hi chain
    # blk3-5 on the packed hi halves; lo result rides delay line 0 from blk3.
    u = UopConfig()
    u.enable_input(InpSel.SRC_0, 1)
    u.enable_input(InpSel.CONST_0, 2)
    u.enable_input(InpSel.CONST_1, 3)
    u.enable_input(InpSel.SRC_1, 4)
    u.enable_input(InpSel.SRC_0_HI, 5)
    u.enable_input(InpSel.SRC_1_HI, 6)
    u.require_inp0 = 1
    u.require_inp1 = 1
    u.trigger = (Trigger.SRC_TENSOR_DONE, Trigger.NONE, Trigger.NONE)
    u.next_uop = (0, 0, 0)
    u.out = {
        OutPath.WR0_LO: OutSel.DELAY_0,
        OutPath.WR0_HI: OutSel.ALU_OUT,
        OutPath.WR1_LO: OutSel.ALU_OUT,
        OutPath.WR1_HI: OutSel.ALU_OUT,
    }
    u.out_enable = {OutPath.WR0_LO: 1, OutPath.WR0_HI: 1,
                    OutPath.WR1_LO: 0, OutPath.WR1_HI: 0}
    CARRY = [DelayInp.PREV_DELAY] * 7

    def blk(aop, s0, s1, delay=None):
        return UopDpConfig(
            op=aop, alu_src0=s0, alu_src1=s1,
            delay=list(delay if delay is not None else CARRY),
            alu_out_enable=1,
            delay_enable=[1, 1, 1, 1, 1, 1, 0],
        )

    dp = [
        blk(AluOp.MULTIPLY, AluInp.PREV_DELAY_0, AluInp.PREV_DELAY_1),
        blk(AluOp.MAX, AluInp.PREV_ALU_OUT, AluInp.PREV_DELAY_2),
        blk(AluOp.MULTIPLY, AluInp.PREV_ALU_OUT, AluInp.PREV_DELAY_3),
        blk(AluOp.MULTIPLY, AluInp.PREV_DELAY_4, AluInp.PREV_DELAY_1,
            delay=[DelayInp.PREV_ALU_OUT] + [DelayInp.PREV_DELAY] * 6),
        blk(AluOp.MAX, AluInp.PREV_ALU_OUT, AluInp.PREV_DELAY_2),
        blk(AluOp.MULTIPLY, AluInp.PREV_ALU_OUT, AluInp.PREV_DELAY_5),
        blk(AluOp.BYPASS, AluInp.PREV_ALU_OUT, AluInp.PREV_ALU_OUT),
        blk(AluOp.BYPASS, AluInp.PREV_ALU_OUT, AluInp.PREV_ALU_OUT),
    ]
    u.datapath_config = dp

    u1x = lower(spec, ver=ver)
    compiled = DveOpSpec(
        name=_QMASK_NAME, opcode=row, uops=u1x, uops_2x=[u],
        perf_max=1, rd1_en=True,
    )
    compiled.validate(ver)
    dops._COMPILE_CACHE[(_QMASK_NAME, ver)] = compiled
    _QMASK_STATE[_QMASK_NAME] = op
    return op


def _qmask_emit(nc, out, in0, s0, s1, in1):
    """out = max(in0*s0, s1) * in1 (s0/s1 per-partition [P,1] APs)."""
    from concourse.bass import dve_ver_for
    from concourse import bass_isa, mybir
    import concourse.dve_ops as dops

    ver = dve_ver_for(nc.trn_type)
    op = _qmask_register(ver)
    vec = nc.vector
    if op.name not in vec.bass.m.ant_custom_dve_ops:
        vec.bass.m.ant_custom_dve_ops = sorted(
            {*vec.bass.m.ant_custom_dve_ops, op.name}
        )
    shape = bass_isa.CustomDveShape.TTSS
    isa_opcode = vec.bass.isa.Opcode[
        f"NEURON_ISA_TPB_OPCODE_CUSTOM_DVE_ANT_{shape.slot()}"
    ].value
    ins = [
        vec.lower_ap(in0, for_isa=True, opt=True),
        vec.lower_ap(in1, for_isa=True, opt=True),
        vec.lower_ap(s0, for_isa=True),
        vec.lower_ap(s1, for_isa=True),
    ]
    outs = [vec.lower_ap(out, for_isa=True, opt=True)]
    return vec.add_instruction(
        bass_isa.InstCustomDveAnt(
            name=vec.bass.get_next_instruction_name(),
            op_name=op.name, rd1_en=True, subdim=0, imm2=0.0,
            shape=shape, row=dops._SUB_OPCODE_FOR_NAME[_QMASK_NAME],
            isa_opcode=isa_opcode, perf_max=1, ins=ins, outs=outs,
        )
    )


_VARSQ_NAME = "VARSQ_ANT"


def _varsq_register(ver):
    if _VARSQ_NAME in _QMASK_STATE:
        return _QMASK_STATE[_VARSQ_NAME]
    import concourse.dve_ops as dops
    from concourse.dve_spec import Spec, Src0, Src1, lower, sq

    spec = Spec(
        body=Src0 - sq(Src1),
        reference=lambda in0, in1, s0, s1, imm2: (
            in0 - in1 * in1
        ).astype(np.float32),
    )
    op = dops.DveOp(name=_VARSQ_NAME, spec=spec, subdim=False, uops_sha={})
    if all(o.name != _VARSQ_NAME for o in dops.OPS):
        dops.OPS.append(op)
    dops.CUSTOM_DVE_SPECS[_VARSQ_NAME] = spec
    if _VARSQ_NAME not in dops._SUB_OPCODE_FOR_NAME:
        row = max(dops._SUB_OPCODE_FOR_NAME.values()) + 1
        assert row < 0x20
        dops._SUB_OPCODE_FOR_NAME[_VARSQ_NAME] = row
    row = dops._SUB_OPCODE_FOR_NAME[_VARSQ_NAME]
    from concourse.dve_uop import DveOpSpec
    compiled = DveOpSpec(
        name=_VARSQ_NAME, opcode=row, uops=lower(spec, ver=ver),
        perf_max=0, rd1_en=True,
    )
    compiled.validate(ver)
    dops._COMPILE_CACHE[(_VARSQ_NAME, ver)] = compiled
    _QMASK_STATE[_VARSQ_NAME] = op
    return op


def _varsq_emit(nc, out, in0, in1):
    """out = in0 - in1*in1 (in0 may be PSUM)."""
    from concourse.bass import dve_ver_for
    from concourse import bass_isa, mybir
    import concourse.dve_ops as dops

    ver = dve_ver_for(nc.trn_type)
    op = _varsq_register(ver)
    vec = nc.vector
    if op.name not in vec.bass.m.ant_custom_dve_ops:
        vec.bass.m.ant_custom_dve_ops = sorted(
            {*vec.bass.m.ant_custom_dve_ops, op.name}
        )
    shape = bass_isa.CustomDveShape.TTSS
    isa_opcode = vec.bass.isa.Opcode[
        f"NEURON_ISA_TPB_OPCODE_CUSTOM_DVE_ANT_{shape.slot()}"
    ].value
    zero = mybir.ImmediateValue(dtype=mybir.dt.float32, value=0.0)
    ins = [
        vec.lower_ap(in0, for_isa=True, opt=True),
        vec.lower_ap(in1, for_isa=True, opt=True),
        zero, zero,
    ]
    outs = [vec.lower_ap(out, for_isa=True, opt=True)]
    return vec.add_instruction(
        bass_isa.InstCustomDveAnt(
            name=vec.bass.get_next_instruction_name(),
            op_name=op.name, rd1_en=True, subdim=0, imm2=0.0,
            shape=shape, row=dops._SUB_OPCODE_FOR_NAME[_VARSQ_NAME],
            isa_opcode=isa_opcode, perf_max=0, ins=ins, outs=outs,
        )
    )


_RELUB_NAME = "RELUB_ANT"


def _relub_register(ver):
    if _RELUB_NAME in _QMASK_STATE:
        return _QMASK_STATE[_RELUB_NAME]
    import concourse.dve_ops as dops
    from concourse.dve_spec import Spec, Src0, C0, lower, relu

    spec = Spec(
        body=relu(Src0 + C0),
        reference=lambda in0, in1, s0, s1, imm2: np.maximum(
            in0 + s0, 0.0
        ).astype(np.float32),
    )
    op = dops.DveOp(name=_RELUB_NAME, spec=spec, subdim=False, uops_sha={})
    if all(o.name != _RELUB_NAME for o in dops.OPS):
        dops.OPS.append(op)
    dops.CUSTOM_DVE_SPECS[_RELUB_NAME] = spec
    if _RELUB_NAME not in dops._SUB_OPCODE_FOR_NAME:
        row = max(dops._SUB_OPCODE_FOR_NAME.values()) + 1
        assert row < 0x20
        dops._SUB_OPCODE_FOR_NAME[_RELUB_NAME] = row
    row = dops._SUB_OPCODE_FOR_NAME[_RELUB_NAME]
    from concourse.dve_uop import DveOpSpec
    compiled = DveOpSpec(
        name=_RELUB_NAME, opcode=row, uops=lower(spec, ver=ver),
        perf_max=0, rd1_en=False,
    )
    compiled.validate(ver)
    dops._COMPILE_CACHE[(_RELUB_NAME, ver)] = compiled
    _QMASK_STATE[_RELUB_NAME] = op
    return op


def _relub_emit(nc, out, in0, s0):
    """out = relu(in0 + s0) on DVE (in0 may be PSUM; s0 per-partition AP)."""
    from concourse.bass import dve_ver_for
    from concourse import bass_isa, mybir
    import concourse.dve_ops as dops

    ver = dve_ver_for(nc.trn_type)
    op = _relub_register(ver)
    vec = nc.vector
    if op.name not in vec.bass.m.ant_custom_dve_ops:
        vec.bass.m.ant_custom_dve_ops = sorted(
            {*vec.bass.m.ant_custom_dve_ops, op.name}
        )
    shape = bass_isa.CustomDveShape.TTSS
    isa_opcode = vec.bass.isa.Opcode[
        f"NEURON_ISA_TPB_OPCODE_CUSTOM_DVE_ANT_{shape.slot()}"
    ].value
    zero = mybir.ImmediateValue(dtype=mybir.dt.float32, value=0.0)
    ins = [
        vec.lower_ap(in0, for_isa=True, opt=True),
        vec.lower_ap(s0, for_isa=True),
        zero,
    ]
    outs = [vec.lower_ap(out, for_isa=True, opt=True)]
    return vec.add_instruction(
        bass_isa.InstCustomDveAnt(
            name=vec.bass.get_next_instruction_name(),
            op_name=op.name, rd1_en=False, subdim=0, imm2=0.0,
            shape=shape, row=dops._SUB_OPCODE_FOR_NAME[_RELUB_NAME],
            isa_opcode=isa_opcode, perf_max=0, ins=ins, outs=outs,
        )
    )


def _build_program():
    import concourse.bacc as bacc
    import concourse.mybir as mybir
    import concourse.tile as tile

    _patch_act_tables()

    F16 = mybir.dt.float16
    F32 = mybir.dt.float32
    AF = mybir.ActivationFunctionType
    OP = mybir.AluOpType

    nc = bacc.Bacc("TRN2", target_bir_lowering=False, debug=False, num_devices=8)

    # ---- I/O ----
    hT = nc.dram_tensor("hT", [128, N], F16, kind="ExternalInput")
    adjT = nc.dram_tensor("adjT", [128, NB * N], F16, kind="ExternalInput")
    # critical pack: [wcat 128 | wadst 8 | wasrep 1024]
    wpackA = nc.dram_tensor("wpackA", [128, 1160], F16, kind="ExternalInput")
    # late pack: [w1 256 | w2 256]
    wpackB = nc.dram_tensor("wpackB", [128, 512], F16, kind="ExternalInput")
    augpk = nc.dram_tensor("augpk", [128, 3072], F16, kind="ExternalInput")
    # packed f32 cols: [b1c 2 | b2c 1 | g1 1 | b1l 1 | g2 1 | b2l 1 | zbias 1 | eps 1]
    wpack32 = nc.dram_tensor("wpack32", [128, 9], F32, kind="ExternalInput")
    sel = nc.dram_tensor("sel", [16, H * 128], F16, kind="ExternalInput")
    e16cat = nc.dram_tensor("e16cat", [1, H * 128], F16, kind="ExternalInput")
    outT = nc.dram_tensor("outT", [128, N], F16, kind="ExternalOutput")

    with tile.TileContext(nc) as tc:
        with (
            tc.tile_pool(name="const", bufs=1) as cpool,
            tc.tile_pool(name="big", bufs=1) as big,
            tc.tile_pool(name="work", bufs=2) as work,
            tc.tile_pool(name="mid", bufs=1) as mid,
            tc.tile_pool(name="rows", bufs=1) as rows,
        ):
            # ---- load everything ----
            # All on the SP (sync) HWDGE ring: FIFO order = priority order.
            hT_t = cpool.tile([128, N], F16)
            nc.sync.dma_start(hT_t[:], hT[:])
            wpA = cpool.tile([128, 1160], F16)
            nc.sync.dma_start(wpA[:], wpackA[:])
            adjq = [
                cpool.tile([128, 4 * N], F16, tag=f"adj{i}", name=f"adj{i}")
                for i in range(2)
            ]
            nc.sync.dma_start(adjq[0][:, 0:2 * N], adjT[:, 0:2 * N])
            nc.sync.dma_start(adjq[0][:, 2 * N:4 * N], adjT[:, 2 * N:4 * N])
            nc.sync.dma_start(adjq[1][:, 0:2 * N], adjT[:, 4 * N:6 * N])
            augt = cpool.tile([128, 3072], F16)
            nc.sync.dma_start(augt[:], augpk[:])
            nc.sync.dma_start(adjq[1][:, 2 * N:4 * N], adjT[:, 6 * N:8 * N])
            wpB = cpool.tile([128, 512], F16)
            nc.sync.dma_start(wpB[:], wpackB[:])
            wp32 = cpool.tile([128, 9], F32)
            nc.sync.dma_start(wp32[:], wpack32[:])
            sel_t = cpool.tile([16, H * 128], F16)
            nc.sync.dma_start(sel_t[:], sel[:])
            e16cat_t = cpool.tile([1, H * 128], F16)
            nc.sync.dma_start(e16cat_t[:], e16cat[:])

            wcat_t = wpA[:, 0:128]
            wadst_t = wpA[:, 128:136]
            wasrep_t = wpA[:, 136:1160]
            w1_t = wpB[:, 0:256]
            w2_t = wpB[:, 256:512]
            aug = augt[:]
            b1_t = wp32[:, 0:2]
            b2_t = wp32[:, 2:3]
            g1_t = wp32[:, 3:4]
            b1l_t = wp32[:, 4:5]
            g2_t = wp32[:, 5:6]
            b2l_t = wp32[:, 6:7]
            zbias = wp32[:, 7:8]
            epsbias = wp32[:, 8:9]

            onescol = cpool.tile([128, 1], F16)
            nc.vector.memset(onescol[:], 1.0)
            jmat = cpool.tile([128, 128], F16)
            nc.vector.memset(jmat[:], 1.0 / 128)
            onesrow = cpool.tile([1, 128], F32)
            nc.vector.memset(onesrow[:], 1.0)

            # ---- phase 1: s-cols(u,v), Wbc, Wh_nat->aug ----
            u_t = [big.tile([128, H], F32, tag=f"u{i}", name=f"u{i}") for i in range(NB)]
            v_t = [big.tile([128, H], F32, tag=f"v{i}", name=f"v{i}") for i in range(NB)]
            wbc = [big.tile([128, N], F16, tag=f"wbc{i}", name=f"wbc{i}") for i in range(H)]
            aug4w = aug.rearrange("p (m h c) -> p m h c", m=NB, h=H, c=48)

            with tc.tile_pool(name="ps1", bufs=3, space="PSUM") as ps1:
                for mb in range(NB):
                    sc_ps = ps1.tile([128, H], F32, tag="ps1")
                    nc.tensor.matmul(
                        sc_ps[:], hT_t[:, mb * 128:(mb + 1) * 128], wadst_t,
                        start=True, stop=True,
                    )
                    nc.scalar.activation(u_t[mb][:], sc_ps[:], AF.Exp, scale=1.0)
                    nc.scalar.activation(v_t[mb][:], sc_ps[:], AF.Exp, scale=0.2)
                    if mb == 0:
                        wb_ps = ps1.tile([128, N], F32, tag="ps1")
                        for ch in range(2):
                            nc.tensor.matmul(
                                wb_ps[:, ch * 512:(ch + 1) * 512],
                                wasrep_t[:, 0:128],
                                hT_t[:, ch * 512:(ch + 1) * 512],
                                start=True, stop=True,
                            )
                        nc.scalar.activation(wbc[0][:], wb_ps[:], AF.Exp, scale=0.8)
                for hh in range(1, H):
                    wb_ps = ps1.tile([128, N], F32, tag="ps1")
                    for ch in range(2):
                        nc.tensor.matmul(
                            wb_ps[:, ch * 512:(ch + 1) * 512],
                            wasrep_t[:, hh * 128:(hh + 1) * 128],
                            hT_t[:, ch * 512:(ch + 1) * 512],
                            start=True, stop=True,
                        )
                    nc.scalar.activation(wbc[hh][:], wb_ps[:], AF.Exp, scale=0.8)
                for mb in range(NB):
                    wn_ps = ps1.tile([128, 128], F32, tag="ps1")
                    nc.tensor.matmul(
                        wn_ps[:], hT_t[:, mb * 128:(mb + 1) * 128], wcat_t,
                        start=True, stop=True,
                    )
                    wn4 = wn_ps[:].rearrange("p (h d) -> p h d", h=H, d=16)
                    nc.scalar.activation(aug4w[:, mb, :, 32:48], wn4[:], AF.Copy)

            # ---- phase 2: attention ----
            stage_all = big.tile([16, H * N], F16)
            with (
                tc.tile_pool(name="ps48", bufs=2, space="PSUM") as ps48,
                tc.tile_pool(name="psacc", bufs=1, space="PSUM") as psacc,
                tc.tile_pool(name="psz", bufs=1, space="PSUM") as psz,
            ):
                asm_ps = psacc.tile([128, N], F32)
                zbc_ps = psz.tile([128, N], F32)
                for hh in range(H):
                    q_half = [
                        work.tile([128, 4 * N], F16, tag="qa", name="qa"),
                        work.tile([128, 4 * N], F16, tag="qb", name="qb"),
                    ]
                    at_ps = ps48.tile([48, N], F32)
                    for half in range(2):
                        qh = q_half[half]
                        for mb in range(half * 4, half * 4 + 4):
                            _qmask_emit(
                                nc, qh[:, (mb % 4) * N:(mb % 4 + 1) * N],
                                wbc[hh][:],
                                u_t[mb][:, hh: hh + 1], v_t[mb][:, hh: hh + 1],
                                adjq[half][:, (mb % 4) * N:(mb % 4 + 1) * N],
                            )
                        for mb in range(half * 4, half * 4 + 4):
                            for ch in range(2):
                                nc.tensor.matmul(
                                    at_ps[:, ch * 512:(ch + 1) * 512],
                                    aug[:, mb * 384 + hh * 48: mb * 384 + hh * 48 + 48],
                                    qh[:, (mb % 4) * N + ch * 512: (mb % 4) * N + ch * 512 + 512],
                                    start=(mb == 0), stop=(mb == NB - 1),
                                )
                    lnz = work.tile([1, N], F32, tag="lnz")
                    nc.scalar.activation(lnz[:], at_ps[0:1, :], AF.Ln, bias=zbias[0:1])
                    zinv_h = work.tile([1, N], F16, tag="zinvh")
                    nc.scalar.activation(zinv_h[:], lnz[:], AF.Exp, scale=-1.0)
                    nc.scalar.activation(
                        stage_all[:, hh * N:(hh + 1) * N], at_ps[32:48, :], AF.Copy
                    )
                    for ch in range(2):
                        nc.tensor.matmul(
                            zbc_ps[:, ch * 512:(ch + 1) * 512],
                            e16cat_t[0:1, hh * 128:(hh + 1) * 128],
                            zinv_h[0:1, ch * 512:(ch + 1) * 512],
                            start=(hh == 0), stop=(hh == H - 1),
                        )
                        nc.tensor.matmul(
                            asm_ps[:, ch * 512:(ch + 1) * 512],
                            sel_t[:, hh * 128:(hh + 1) * 128],
                            stage_all[:, hh * N + ch * 512: hh * N + ch * 512 + 512],
                            start=(hh == 0), stop=(hh == H - 1),
                        )

                zbcf = big.tile([128, N], F16)
                nc.scalar.activation(zbcf[:, 0:512], zbc_ps[:, 0:512], AF.Copy)
                nc.scalar.activation(zbcf[:, 512:1024], zbc_ps[:, 512:1024],
                                     AF.Copy)
                hh_t = big.tile([128, N], F16)
                x_res = big.tile([128, N], F16)
                for c2 in range(2):
                    nc.vector.tensor_tensor(
                        hh_t[:, c2 * 512:(c2 + 1) * 512],
                        asm_ps[:, c2 * 512:(c2 + 1) * 512],
                        zbcf[:, c2 * 512:(c2 + 1) * 512], op=OP.mult)
                    nc.vector.tensor_tensor(
                        x_res[:, c2 * 512:(c2 + 1) * 512],
                        hh_t[:, c2 * 512:(c2 + 1) * 512],
                        hT_t[:, c2 * 512:(c2 + 1) * 512], op=OP.add)

            with tc.tile_pool(name="ps3", bufs=2, space="PSUM") as ps3:
                # ---- chunked epilogue: normalize+residual, LN1, FFN, LN2 ----
                C = 512

                def cs(t, c):
                    return t[:, c * C:(c + 1) * C]

                def layernorm_T(x_in, g_col, b_col, out_tile, ps_pool, nm):
                    """Column-chunked transposed layernorm; J=ones/128 matmul
                    produces mean / mean-square directly as broadcast tiles."""
                    x2 = mid.tile([128, N], F16, tag=f"x2{nm}")
                    for c in range(N // C):
                        nc.vector.tensor_tensor(
                            cs(x2, c), cs(x_in, c), cs(x_in, c), op=OP.mult
                        )
                    for c in range(N // C):
                        mu_ps = ps_pool.tile([128, C], F32, tag="psb")
                        ssq_ps = ps_pool.tile([128, C], F32, tag="psb")
                        nc.tensor.matmul(mu_ps[:], jmat[:], cs(x_in, c),
                                         start=True, stop=True)
                        nc.tensor.matmul(ssq_ps[:], jmat[:], cs(x2, c),
                                         start=True, stop=True)
                        mu_bc = mid.tile([128, C], F16, tag=f"mbc{nm}{c}")
                        nc.scalar.activation(mu_bc[:], mu_ps[:], AF.Copy)
                        var = mid.tile([128, C], F16, tag=f"va{nm}{c}")
                        _varsq_emit(nc, var[:], ssq_ps[:], mu_bc[:])
                        lnv = mid.tile([128, C], F16, tag=f"lv{nm}{c}")
                        nc.scalar.activation(lnv[:], var[:], AF.Ln, bias=epsbias)
                        rstd = mid.tile([128, C], F16, tag=f"rs{nm}{c}")
                        nc.scalar.activation(rstd[:], lnv[:], AF.Exp, scale=-0.5)
                        t_ = mid.tile([128, C], F16, tag=f"lnt{nm}{c}")
                        nc.vector.tensor_tensor(t_[:], cs(x_in, c), mu_bc[:],
                                                op=OP.subtract)
                        xn = mid.tile([128, C], F16, tag=f"lnxn{nm}{c}")
                        nc.vector.tensor_tensor(xn[:], t_[:], rstd[:], op=OP.mult)
                        nc.vector.tensor_scalar(
                            cs(out_tile, c), xn[:], g_col[:], b_col[:],
                            op0=OP.mult, op1=OP.add,
                        )

                xc = big.tile([128, N], F16)
                y1s = big.tile([128, 2 * N], F16)
                y2b = big.tile([128, N], F16)
                z_res = big.tile([128, N], F16)
                outT_sb = big.tile([128, N], F16)

                layernorm_T(x_res, g1_t, b1l_t, xc, ps3, "a")

                # FFN (chunked)
                for cb in range(2):
                    y1_ps = ps3.tile([128, N], F32, tag="ps3")
                    for c in range(N // C):
                        nc.tensor.matmul(
                            cs(y1_ps, c), w1_t[:, cb * 128:(cb + 1) * 128],
                            cs(xc, c), start=True, stop=True,
                        )
                        nc.scalar.activation(
                            y1s[:, cb * N + c * C: cb * N + (c + 1) * C],
                            cs(y1_ps, c), AF.Relu, bias=b1_t[:, cb:cb + 1],
                        )
                y2_ps = ps3.tile([128, N], F32, tag="ps3")
                for cb in range(2):
                    for c in range(N // C):
                        nc.tensor.matmul(
                            cs(y2_ps, c), w2_t[:, cb * 128:(cb + 1) * 128],
                            y1s[:, cb * N + c * C: cb * N + (c + 1) * C],
                            start=(cb == 0), stop=(cb == 1),
                        )
                for c in range(N // C):
                    nc.vector.scalar_tensor_tensor(
                        cs(z_res, c), cs(y2_ps, c), b2_t, cs(xc, c),
                        op0=OP.add, op1=OP.add,
                    )
                layernorm_T(z_res, g2_t, b2l_t, outT_sb, ps3, "b")
                for c in range(N // C):
                    nc.sync.dma_start(outT[:, c * C:(c + 1) * C],
                                      outT_sb[:, c * C:(c + 1) * C])

    nc.compile()
    return nc


def _host_prep(h, adj_mask, W, a, ln1_g, ln1_b, w1, b1, w2, b2, ln2_g, ln2_b):
    f16 = np.float16
    f32 = np.float32
    wcat = np.ascontiguousarray(
        np.transpose(np.asarray(W, f32), (1, 0, 2)).reshape(128, 128)
    ).astype(f16)
    a = np.asarray(a, f32)
    a_src, a_dst = a[:, :HD], a[:, HD:]
    Wf = np.asarray(W, f32)
    wa_dst = np.einsum("hid,hd->ih", Wf, a_dst).astype(f16)
    wa_src = np.einsum("hid,hd->ih", Wf, a_src)
    wasrep = np.repeat(wa_src[:, :, None], 128, axis=2).reshape(128, H * 128).astype(f16)
    sel = np.zeros((16, H * 128), f16)
    for hh in range(H):
        sel[np.arange(16), hh * 128 + hh * 16 + np.arange(16)] = 1.0
    e16cat = np.zeros((1, H * 128), f16)
    for hh in range(H):
        e16cat[0, hh * 128 + hh * 16: hh * 128 + (hh + 1) * 16] = 1.0
    w1c = np.asarray(w1, f32).astype(f16)
    w2c = np.ascontiguousarray(
        np.asarray(w2, f32).reshape(2, 128, 128).transpose(1, 0, 2).reshape(128, 256)
    ).astype(f16)
    augs = np.zeros((128, NB * 384), f16)
    augs[:, np.arange(NB * H) * 48] = 1.0  # ones columns
    wpackA = np.concatenate([wcat, wa_dst, wasrep], axis=1)
    wpackB = np.concatenate([w1c, w2c], axis=1)

    wpack32 = np.zeros((128, 9), f32)
    wpack32[:, 0:2] = np.asarray(b1, f32).reshape(2, 128).T
    wpack32[:, 2] = np.asarray(b2, f32)
    wpack32[:, 3] = np.asarray(ln1_g, f32)
    wpack32[:, 4] = np.asarray(ln1_b, f32)
    wpack32[:, 5] = np.asarray(ln2_g, f32)
    wpack32[:, 6] = np.asarray(ln2_b, f32)
    wpack32[:, 7] = 1e-4
    wpack32[:, 8] = EPS

    shared = dict(wpackA=wpackA, wpackB=wpackB, augpk=augs, wpack32=wpack32,
                  sel=sel, e16cat=e16cat)

    h = np.asarray(h, f32)
    adj = np.asarray(adj_mask)
    in_maps = []
    for b in range(B):
        hT = np.ascontiguousarray(h[b].T).astype(f16)
        adjT = np.ascontiguousarray(
            (adj[b] != 0).T.astype(f16).reshape(NB, 128, N).transpose(1, 0, 2).reshape(128, NB * N)
        )
        in_maps.append(dict(hT=hT, adjT=adjT, **shared))
    return in_maps


def kernel(**inputs):
    from concourse.bass_utils import run_bass_kernel_spmd

    if "nc" not in _CACHE:
        _CACHE["nc"] = _build_program()
    nc = _CACHE["nc"]

    in_maps = _host_prep(**inputs)
    res = run_bass_kernel_spmd(nc, in_maps, list(range(B)))
    out = np.empty((B, N, OUT_DIM), np.float32)
    for b in range(B):
        out[b] = res.results[b]["outT"].T
    return out

